# revision 12
# baseline (speedup 1.0000x reference)
"""Multi-head attention (B=2, S=2048, D=1024, H=16, RoPE) on 8 Trainium2 cores.

Sharding: tensor-parallel over heads. Core c owns heads (2c, 2c+1):
 - W_qkv column-sliced to that head pair (q|k|v blocks of 128 cols each),
 - W_out row-sliced to the pair's 128 input dims,
 - every core reads all tokens (x shipped pre-transposed as x^T),
 - each core emits a partial [4096, 1024] output; host sums the 8 partials
   and adds b_out (the Megatron-style allreduce done on host).

Device program: v1's attention loop (pop-every-tb out-projection
deferral, avmm trailing by one key block, s_t SBUF staging, per-head
DRAM-bounce reciprocal) with a faster front end:
 - all matmul operands are f32r tiles filled by DMA directly (dram
   tensors declared f32r; no DVE casts anywhere),
 - the rotate-half matrix, identity, V2 skeleton (ones/zero columns)
   and kT zero pads ship from the host as constants; startup DMAs are
   split across the sync and scalar HWDGE queues so the first qkv
   matmul issues at ~5us,
 - qkv bias is applied as a per-partition tensor_scalar during PSUM
   evacuation (no ones-row bias matmuls).
"""

import sys

if "/opt/trn_rl_repo" not in sys.path:
    sys.path.insert(0, "/opt/trn_rl_repo")

import numpy as np

import concourse.bacc as bacc
import concourse.mybir as mybir
from concourse.tile import TileContext
from concourse.bass_utils import run_bass_kernel_spmd

F32 = mybir.dt.float32
F32R = mybir.dt.float32r
ADD = mybir.AluOpType.add
MUL = mybir.AluOpType.mult
EXP = mybir.ActivationFunctionType.Exp

B, S, D, H, DH = 2, 2048, 1024, 16, 64
S2 = B * S              # 4096 tokens total
CH = 256                # token chunk for the projection phase
CPB = S // CH           # 8 chunks per batch
NSC = 4                 # 512-query chunks per batch
NTB = S // 128          # 16 key blocks per batch
VG = 193                # V2 group: VA(64) | 1 | zeros(63) | 1 | VB(64)


def _build_program():
    nc = bacc.Bacc("TRN2", target_bir_lowering=False, debug=False, num_devices=8)

    xT = nc.dram_tensor("xT", [D, S2], F32R, kind="ExternalInput")
    W = nc.dram_tensor("W", [D, 384], F32R, kind="ExternalInput")
    bqc_d = nc.dram_tensor("bqc", [128, 3], F32, kind="ExternalInput")
    Wo = nc.dram_tensor("Wo", [128, 1024], F32R, kind="ExternalInput")
    ctab_d = nc.dram_tensor("ctab", [128, S], F32, kind="ExternalInput")
    stab_d = nc.dram_tensor("stab", [128, S], F32, kind="ExternalInput")
    p2_d = nc.dram_tensor("p2", [128, 128], F32R, kind="ExternalInput")
    idn_d = nc.dram_tensor("idn", [128, 128], F32, kind="ExternalInput")
    vcon_d = nc.dram_tensor("vcon", [128, NTB * VG], F32R, kind="ExternalInput")
    zer_d = nc.dram_tensor("zer", [64, S], F32R, kind="ExternalInput")
    out_d = nc.dram_tensor("out", [S2, D], F32, kind="ExternalOutput")

    xT_re = xT.rearrange("(kb p) n -> p kb n", p=128)   # [128, 8, 4096]
    W_re = W.rearrange("(kb p) m -> p kb m", p=128)     # [128, 8, 384]

    with TileContext(nc) as tc:
        with tc.tile_pool(name="consts", bufs=1) as cp, \
             tc.tile_pool(name="xc", bufs=2) as xcp, \
             tc.tile_pool(name="pre", bufs=4) as prep, \
             tc.tile_pool(name="tmp", bufs=4) as tmpp, \
             tc.tile_pool(name="vtc", bufs=2) as vtcp, \
             tc.tile_pool(name="pt", bufs=3) as ptp, \
             tc.tile_pool(name="mrgs", bufs=3) as mrgs, \
             tc.tile_pool(name="lt", bufs=2) as ltp, \
             tc.tile_pool(name="a2c", bufs=2) as a2cp, \
             tc.tile_pool(name="osb", bufs=2) as osbp, \
             tc.tile_pool(name="dram", bufs=4, space="DRAM") as drp, \
             tc.tile_pool(name="ps512", bufs=2, space="PSUM") as ps512, \
             tc.tile_pool(name="psscore", bufs=2, space="PSUM") as pssc, \
             tc.tile_pool(name="psacc", bufs=1, space="PSUM") as psacc:
            ident = cp.tile([128, 128], F32, tag="ident")
            P2r = cp.tile([128, 128], F32R, tag="P2r")
            bq_c = cp.tile([128, 3], F32, tag="bq_c")
            ctab = cp.tile([128, S], F32, tag="ctab")
            stab = cp.tile([128, S], F32, tag="stab")
            W_r = cp.tile([128, 8 * 384], F32R, tag="W_r")
            Wo_r = cp.tile([128, 1024], F32R, tag="Wo_r")
            qTb = [cp.tile([128, S], F32R, name=f"qT{b}", tag=f"qT{b}")
                   for b in range(B)]
            kTab = [cp.tile([128, S], F32R, name=f"kTa{b}", tag=f"kTa{b}")
                    for b in range(B)]   # head A rows 0:64, rows 64:128 zero
            kTbb = [cp.tile([128, S], F32R, name=f"kTb{b}", tag=f"kTb{b}")
                    for b in range(B)]   # head B rows 64:128, rows 0:64 zero
            V2b = [cp.tile([128, NTB * VG], F32R, name=f"V2{b}", tag=f"V2{b}")
                   for b in range(B)]

            # Startup loads.  Scalar HWDGE queue: small PE constants first
            # (the first chunk's rope/transpose gate on them), then the big
            # weight/table loads in order of first use.  vcon is the FULL V2
            # skeleton shipped as one contiguous DMA (a column-strided
            # version generates 2048 tiny descriptors and takes ~30us).
            nc.scalar.dma_start(out=P2r[:], in_=p2_d[:])
            nc.scalar.dma_start(out=ident[:], in_=idn_d[:])
            nc.scalar.dma_start(
                out=W_r[:].rearrange("p (kb m) -> p kb m", kb=8),
                in_=W_re[:, :, :])
            nc.scalar.dma_start(out=V2b[0][:], in_=vcon_d[:])
            nc.scalar.dma_start(out=ctab[:], in_=ctab_d[:])
            nc.scalar.dma_start(out=stab[:], in_=stab_d[:])
            nc.scalar.dma_start(out=Wo_r[:], in_=Wo[:])
            nc.scalar.dma_start(out=kTab[1][64:128, :], in_=zer_d[:])
            nc.scalar.dma_start(out=kTbb[1][0:64, :], in_=zer_d[:])
            nc.scalar.dma_start(out=V2b[1][:], in_=vcon_d[:])

            # sync queue: the tiny bias + batch-0 pads, then the x chunks
            nc.sync.dma_start(out=bq_c[:], in_=bqc_d[:])
            nc.sync.dma_start(out=kTab[0][64:128, :], in_=zer_d[:])
            nc.sync.dma_start(out=kTbb[0][0:64, :], in_=zer_d[:])

            # ---------------- emitters ----------------------------------
            def emit_chunk(ch):
                bb, cb = ch // CPB, ch % CPB
                scol = cb * CH
                xc = xcp.tile([128, 8 * CH], F32R, tag="xc", name=f"xc{ch}")
                nc.sync.dma_start(
                    out=xc[:].rearrange("p (kb n) -> p kb n", kb=8),
                    in_=xT_re[:, :, ch * CH:(ch + 1) * CH])

                ps3 = []
                for mt in range(3):     # q, k, v
                    ps = ps512.tile([128, CH], F32, tag="ps512",
                                    name=f"qkv{ch}_{mt}")
                    for kb in range(8):
                        nc.tensor.matmul(
                            ps[:],
                            W_r[:, kb * 384 + mt * 128:kb * 384 + (mt + 1) * 128],
                            xc[:, kb * CH:(kb + 1) * CH],
                            start=(kb == 0), stop=(kb == 7))
                    ps3.append(ps)

                # rope for q and k (bias folded into the PSUM evacuation)
                for mt in (0, 1):
                    pre = prep.tile([128, CH], F32R, tag="pre",
                                    name=f"pre{ch}_{mt}")
                    nc.vector.tensor_scalar(
                        out=pre[:], in0=ps3[mt][:],
                        scalar1=bq_c[:, mt:mt + 1], scalar2=None, op0=ADD)
                    rot = ps512.tile([128, CH], F32, tag="ps512",
                                     name=f"rot{ch}_{mt}")
                    nc.tensor.matmul(rot[:], P2r[:], pre[:], start=True, stop=True)
                    t1 = tmpp.tile([128, CH], F32, tag="tmp", name=f"t1_{ch}_{mt}")
                    nc.vector.tensor_tensor(
                        out=t1[:], in0=rot[:], in1=stab[:, scol:scol + CH], op=MUL)
                    t2 = tmpp.tile([128, CH], F32, tag="tmp", name=f"t2_{ch}_{mt}")
                    nc.vector.tensor_tensor(
                        out=t2[:], in0=pre[:], in1=ctab[:, scol:scol + CH], op=MUL)
                    csl = slice(scol, scol + CH)
                    if mt == 0:
                        nc.vector.tensor_tensor(
                            out=qTb[bb][:, csl], in0=t1[:], in1=t2[:], op=ADD)
                    else:
                        nc.vector.tensor_tensor(
                            out=kTab[bb][0:64, csl],
                            in0=t1[0:64, :], in1=t2[0:64, :], op=ADD)
                        nc.vector.tensor_tensor(
                            out=kTbb[bb][64:128, csl],
                            in0=t1[64:128, :], in1=t2[64:128, :], op=ADD)

                # V: evac (+bias) + PE transpose into [t, d]
                nt = CH // 128
                vt = vtcp.tile([128, CH], F32, tag="vt", name=f"vt{ch}")
                nc.vector.tensor_scalar(
                    out=vt[:], in0=ps3[2][:],
                    scalar1=bq_c[:, 2:3], scalar2=None, op0=ADD)
                v2p = ps512.tile([128, CH], F32, tag="ps512", name=f"v2p{ch}")
                for i in range(nt):
                    nc.tensor.transpose(
                        v2p[:, i * 128:(i + 1) * 128],
                        vt[:, i * 128:(i + 1) * 128], ident[:])
                g0 = cb * nt
                dst = V2b[bb][:, g0 * VG:(g0 + nt) * VG].rearrange(
                    "p (i c) -> p i c", i=nt)
                src = v2p[:].rearrange("p (i h d) -> p i h d", i=nt, h=2)
                # head A -> cols 0:64, head B -> cols 129:193 of each group
                nc.vector.tensor_copy(dst[:, :, 0:64], src[:, :, 0:1, :])
                nc.vector.tensor_copy(dst[:, :, 129:193], src[:, :, 1:2, :])

            pending_out = []

            def emit_sc(bb, sc):
                qcol = sc * 512
                qT, kTa, kTb, V2 = qTb[bb], kTab[bb], kTbb[bb], V2b[bb]
                gam = psacc.tile([128, 1024], F32, tag="acc",
                                 name=f"gam{bb}_{sc}")

                def av_mms(tb, pa):
                    gcol = tb * VG
                    nc.tensor.matmul(
                        gam[0:65, 0:512],
                        V2[:, gcol:gcol + 65], pa[:, 0:512],
                        start=(tb == 0), stop=(tb == NTB - 1))
                    nc.tensor.matmul(
                        gam[:, 512:1024],
                        V2[:, gcol + 65:gcol + 193], pa[:, 512:1024],
                        start=(tb == 0), stop=(tb == NTB - 1))

                prev = None
                for tb in range(NTB):
                    tcol = tb * 128
                    sco = pssc.tile([128, 1024], F32, tag="score",
                                    name=f"sco{bb}_{sc}_{tb}")
                    for h, kt in ((0, kTa), (1, kTb)):
                        nc.tensor.matmul(
                            sco[:, 512 * h:512 * (h + 1)],
                            kt[:, tcol:tcol + 128], qT[:, qcol:qcol + 512],
                            start=True, stop=True)
                    pa = ptp.tile([128, 1024], F32R, tag="pt",
                                  name=f"pa{bb}_{sc}_{tb}")
                    nc.scalar.activation(pa[:], sco[:], EXP, scale=0.125)
                    if prev is not None:
                        av_mms(*prev)
                    if pending_out:
                        pending_out.pop(0)()
                    prev = (tb, pa)
                av_mms(*prev)

                # merge + divide: head A rows 0:63 (l at row 64 of gam-A),
                # head B rows 64:127 (l at row 63 of gam-B)
                a2 = a2cp.tile([128, 512], F32R, tag="a2c", name=f"a2c{bb}_{sc}")
                for h in range(2):
                    s_t = mrgs.tile([128, 512], F32, tag="s_t",
                                    name=f"s_t{bb}_{sc}_{h}")
                    if h == 0:
                        nc.gpsimd.memset(s_t[64:128, :], 0.0)
                        nc.vector.tensor_copy(s_t[0:65, :], gam[0:65, 0:512])
                        lrow = s_t[64:65, :]
                    else:
                        nc.vector.tensor_copy(s_t[:], gam[:, 512:1024])
                        lrow = s_t[63:64, :]
                    lscr = drp.tile([512], F32, tag="lscr",
                                    name=f"ls{bb}_{sc}_{h}")
                    nc.sync.dma_start(out=lscr[None, :], in_=lrow)
                    l4 = ltp.tile([128, 4], F32, tag="l4", name=f"l4_{bb}{sc}{h}")
                    nc.sync.dma_start(
                        out=l4[:], in_=lscr[:].rearrange("(p f) -> p f", p=128))
                    r4 = ltp.tile([128, 4], F32, tag="r4", name=f"r4_{bb}{sc}{h}")
                    nc.vector.reciprocal(r4[:], l4[:])
                    rscr = drp.tile([512], F32, tag="rscr",
                                    name=f"rs{bb}_{sc}_{h}")
                    nc.sync.dma_start(
                        out=rscr[:].rearrange("(p f) -> p f", p=128), in_=r4[:])
                    rl1 = ltp.tile([128, 512], F32, tag="rl1",
                                   name=f"rl1_{bb}{sc}{h}")
                    nc.sync.dma_start(out=rl1[0:1, :], in_=rscr[None, :])
                    rlb = ltp.tile([128, 512], F32, tag="rlb",
                                   name=f"rlb_{bb}{sc}{h}")
                    nc.gpsimd.partition_broadcast(out_ap=rlb[:], in_ap=rl1[0:1, :])
                    if h == 0:
                        nc.vector.tensor_tensor(
                            out=a2[0:64, :], in0=s_t[0:64, :],
                            in1=rlb[0:64, :], op=MUL)
                    else:
                        nc.vector.tensor_tensor(
                            out=a2[64:128, :], in0=s_t[64:128, :],
                            in1=rlb[64:128, :], op=MUL)

                def make_outproj(bb, qcol, a2):
                    def emit_nb(nb):
                        o = osbp.tile([128, 1024], F32, tag="osb",
                                      name=f"osb{bb}_{qcol}_{nb}")
                        for jc in range(2):
                            om = ps512.tile([128, 512], F32, tag="ps512",
                                            name=f"om{bb}_{qcol}_{nb}_{jc}")
                            nc.tensor.matmul(
                                om[:], a2[:, nb * 128:(nb + 1) * 128],
                                Wo_r[:, jc * 512:(jc + 1) * 512],
                                start=True, stop=True)
                            nc.vector.tensor_copy(
                                o[:, jc * 512:(jc + 1) * 512], om[:])
                        nc.sync.dma_start(
                            out=out_d[bb * S + qcol + nb * 128:
                                      bb * S + qcol + (nb + 1) * 128, :],
                            in_=o[:])
                    return [lambda nb=nb: emit_nb(nb) for nb in range(4)]

                pending_out.extend(make_outproj(bb, qcol, a2))

            # ---------------- schedule ----------------------------------
            for ch in range(CPB):           # batch 0 projections
                emit_chunk(ch)
            for sc in range(NSC):           # batch 0 attention || batch 1 proj
                emit_sc(0, sc)
                emit_chunk(CPB + 2 * sc)
                emit_chunk(CPB + 2 * sc + 1)
            for sc in range(NSC):           # batch 1 attention
                emit_sc(1, sc)
            for fn in pending_out:
                fn()

    nc.compile()
    return nc


_PROG = None


def _get_program():
    global _PROG
    if _PROG is None:
        _PROG = _build_program()
    return _PROG


def _rope_tables():
    inv_freq = (1.0 / (10000.0 ** (np.arange(0, DH, 2, dtype=np.float32) / DH)))
    invf2 = inv_freq[np.arange(128) % 32]
    ang = np.arange(S, dtype=np.float32)[None, :] * invf2[:, None].astype(np.float32)
    return (np.cos(ang).astype(np.float32), np.sin(ang).astype(np.float32))


def make_in_maps(x, W_qkv, b_qkv, W_out, b_out):
    x = np.asarray(x, dtype=np.float32)
    W_qkv = np.asarray(W_qkv, dtype=np.float32)
    b_qkv = np.asarray(b_qkv, dtype=np.float32)
    W_out = np.asarray(W_out, dtype=np.float32)

    xT = np.ascontiguousarray(x.reshape(S2, D).T)
    ct, st = _rope_tables()

    # P2[k, j]: rotate-half matrix so that (P2^T q)[j] = -q[j+32] for
    # (j%64)<32 and +q[j-32] otherwise (per 64-dim head half).
    p2 = np.zeros((128, 128), dtype=np.float32)
    for k in range(128):
        j = k ^ 32
        p2[k, j] = -1.0 if (k % 64) >= 32 else 1.0
    idn = np.eye(128, dtype=np.float32)
    # V2 skeleton: per group [VA(0:64) | 1(64) | zeros(65:128) | 1(128) |
    # VB(129:193)] -- head A's l lands at gam row 64, head B's at row 63.
    vcon = np.zeros((128, NTB, VG), dtype=np.float32)
    vcon[:, :, 64] = 1.0
    vcon[:, :, 128] = 1.0
    vcon = np.ascontiguousarray(vcon.reshape(128, NTB * VG))
    zer = np.zeros((64, S), dtype=np.float32)

    in_maps = []
    for c in range(8):
        hA, hB = 2 * c, 2 * c + 1
        cols = np.r_[hA * DH:(hA + 1) * DH, hB * DH:(hB + 1) * DH]
        Wc = np.ascontiguousarray(
            np.concatenate([W_qkv[:, off + cols] for off in (0, D, 2 * D)], axis=1))
        bqc = np.ascontiguousarray(
            np.stack([b_qkv[off + cols] for off in (0, D, 2 * D)], axis=1))
        Woc = np.ascontiguousarray(W_out[c * 128:(c + 1) * 128, :])
        in_maps.append(
            {"xT": xT, "W": Wc, "bqc": bqc, "Wo": Woc, "ctab": ct, "stab": st,
             "p2": p2, "idn": idn, "vcon": vcon, "zer": zer})
    return in_maps


def assemble_output(results, b_out):
    acc = results[0]["out"].astype(np.float64)
    for c in range(1, 8):
        acc += results[c]["out"]
    out = acc + np.asarray(b_out, dtype=np.float64)
    return out.reshape(B, S, D).astype(np.float32)


def kernel(x, W_qkv, b_qkv, W_out, b_out):
    nc = _get_program()
    in_maps = make_in_maps(x, W_qkv, b_qkv, W_out, b_out)
    res = run_bass_kernel_spmd(nc, in_maps, core_ids=list(range(8)))
    return assemble_output(res.results, b_out)


if __name__ == "__main__":
    rng = np.random.default_rng(0)
    ins = {
        "x": rng.standard_normal((B, S, D), dtype=np.float32),
        "W_qkv": rng.standard_normal((D, 3 * D), dtype=np.float32) / 32.0,
        "b_qkv": np.zeros(3 * D, np.float32),
        "W_out": rng.standard_normal((D, D), dtype=np.float32) / 32.0,
        "b_out": np.zeros(D, np.float32),
    }
    o = kernel(**ins)
    print("kernel ran:", o.shape, o.dtype)
